# revision 4
# baseline (speedup 1.0000x reference)
"""Trainium2 Bass kernel for nn_CrossTransformerBlock (sparse kNN cross-attention).

Shapes (hardcoded): B=4, NQ=4096, N=2048, DIM=128, DG=256, DI=256, K=16.
Sharding: 8 cores = (batch b, query-half h); each core handles 2048 queries
against its batch's 2048 points.

Per-core pipeline (software-pipelined, 2-block lookahead):
  prep:  PE projects k2~ = -(W_g1 W_k)@pts, v' = W_v@pts + b_d2, g = W_d1@xyz,
         u = W_d1@xyz_q + b_d1; PE-transposes (k2~|v'|g) into a row-major DRAM
         table [2048, 384] f16.
  topk:  PE computes s = 2 q.x - |x|^2 (f32r) -> PSUM; DVE ORs the chunk-local
         index into the mantissa low 8 bits, 8x InstMax (top-8 per 256-chunk)
         + match_replace merge -> top-16 packed values; idx = (v&0xFF)+chunk*256.
  gather: selector matmul rearranges idx into the wrapped int16 layout;
         dma_gather(transpose=True) pulls 16 rows/query into SBUF [128f,3,4096].
  mlp:   h=relu(u-g) on DVE; vpos = W_d2@h + I@v' on PE (PSUM accumulate);
         g1 = relu(M@h + I@k2~ + bias) with M = W_g1@W_d2 (constants folded
         into the ACT evacuation bias); expt = exp(W_g2@g1 + b_g2);
         wp = expt*vpos on DVE; neighbor sums on PE as 16 accumulating
         identity matmuls (+global slot via broadcast tiles); DVE recip+mult.
"""

import numpy as np

import concourse.bass as bass
import concourse.bacc as bacc
import concourse.mybir as mybir
from concourse.tile import TileContext
from concourse.bass_utils import run_bass_kernel_spmd

F32 = mybir.dt.float32
F16 = mybir.dt.float16
U32 = mybir.dt.uint32
I16 = mybir.dt.int16
ALU = mybir.AluOpType
ACTF = mybir.ActivationFunctionType

B, NQ, N, DIM, DG, DI, K = 4, 4096, 2048, 128, 256, 256, 16
NQC = 2048          # queries per core
QTILE = 128         # topk tile (queries on partitions)
NTILES = NQC // QTILE
QBLK = 256          # gather/MLP block
NBLK = NQC // QBLK
CHUNK = 256         # topk candidate chunk (top-8 per chunk)
NCHUNK = N // CHUNK
ROWF = 3 * DIM      # table row features (k2~|v'|g)

_CACHE = {}


def _build(stage='full'):
    nc = bacc.Bacc("TRN2", target_bir_lowering=False, debug=False, num_devices=8)

    # ---- external inputs (per core) ----
    qx4 = nc.dram_tensor("qx4", [4, NQC], F32, kind="ExternalInput")
    xt4 = nc.dram_tensor("xt4", [4, N], F32, kind="ExternalInput")
    ptsT = nc.dram_tensor("ptsT", [128, 2 * N], F16, kind="ExternalInput")
    xyzq4 = nc.dram_tensor("xyzq4", [4, NQC], F16, kind="ExternalInput")
    xyzn4 = nc.dram_tensor("xyzn4", [4, N], F16, kind="ExternalInput")
    wck_l = nc.dram_tensor("wck_l", [128, 2 * DIM], F16, kind="ExternalInput")
    wv_l = nc.dram_tensor("wv_l", [128, 2 * DIM], F16, kind="ExternalInput")
    wd1_l = nc.dram_tensor("wd1_l", [4, DIM], F16, kind="ExternalInput")
    wd2_l = nc.dram_tensor("wd2_l", [DIM, DIM], F16, kind="ExternalInput")
    wm_l = nc.dram_tensor("wm_l", [DIM, DIM], F16, kind="ExternalInput")
    wg2_l = nc.dram_tensor("wg2_l", [DIM, DIM], F16, kind="ExternalInput")
    # per-partition column vectors [128, c] f32: b_d2, bg1eff, b_g2
    colv = nc.dram_tensor("colv", [DIM, 3], F32, kind="ExternalInput")
    # broadcast rows [128, 512] f16: (eg | egv)
    ebt = nc.dram_tensor("ebt", [128, 2 * QBLK], F16, kind="ExternalInput")
    # constants
    esel = nc.dram_tensor("esel", [128, 128], F16, kind="ExternalInput")
    masks = nc.dram_tensor("masks", [128, 2 * 256], F16, kind="ExternalInput")
    ident16 = nc.dram_tensor("ident16", [128, 2 * 128], F16, kind="ExternalInput")
    ident32 = nc.dram_tensor("ident32", [128, 128], F32, kind="ExternalInput")

    out = nc.dram_tensor("out", [NQC, DIM], F32, kind="ExternalOutput")

    with TileContext(nc) as tc:
        with tc.tile_pool(name="const", bufs=1) as cpool, \
             tc.tile_pool(name="upool", bufs=1) as upool, \
             tc.tile_pool(name="work", bufs=2) as work, \
             tc.tile_pool(name="spkp", bufs=1) as spkp, \
             tc.tile_pool(name="gp", bufs=2) as gp, \
             tc.tile_pool(name="mlp", bufs=2) as mlp, \
             tc.tile_pool(name="psd", bufs=2, space="PSUM") as psd, \
             tc.tile_pool(name="psm", bufs=2, space="PSUM") as psm, \
             tc.tile_pool(name="pse", bufs=1, space="PSUM") as pse, \
             tc.tile_pool(name="pss", bufs=1, space="PSUM") as pss, \
             tc.tile_pool(name="dram", bufs=1, space="DRAM") as dpool:

            # ---------- load constants / operands ----------
            qx4_s = cpool.tile([4, NQC], F32)
            xt4_s = cpool.tile([4, N], F32)
            nc.sync.dma_start(qx4_s[:], qx4[:])
            nc.sync.dma_start(xt4_s[:], xt4[:])
            xyzq4_s = cpool.tile([4, NQC], F16)
            xyzn4_s = cpool.tile([4, N], F16)
            nc.sync.dma_start(xyzq4_s[:], xyzq4[:])
            nc.sync.dma_start(xyzn4_s[:], xyzn4[:])
            wck_s = cpool.tile([128, 2 * DIM], F16)
            wv_s = cpool.tile([128, 2 * DIM], F16)
            nc.sync.dma_start(wck_s[:], wck_l[:])
            nc.sync.dma_start(wv_s[:], wv_l[:])
            wd1_s = cpool.tile([4, DIM], F16)
            nc.sync.dma_start(wd1_s[:], wd1_l[:])
            wd2_s = cpool.tile([DIM, DIM], F16)
            wm_s = cpool.tile([DIM, DIM], F16)
            wg2_s = cpool.tile([DIM, DIM], F16)
            nc.sync.dma_start(wd2_s[:], wd2_l[:])
            nc.sync.dma_start(wm_s[:], wm_l[:])
            nc.sync.dma_start(wg2_s[:], wg2_l[:])
            colv_s = cpool.tile([DIM, 3], F32)
            nc.sync.dma_start(colv_s[:], colv[:])
            b_d2 = colv_s[:, 0:1]
            bg1eff = colv_s[:, 1:2]
            b_g2 = colv_s[:, 2:3]
            ebt_s = cpool.tile([128, 2 * QBLK], F16)
            nc.sync.dma_start(ebt_s[:], ebt[:])
            esel_s = cpool.tile([128, 128], F16)
            nc.sync.dma_start(esel_s[:], esel[:])
            masks_s = cpool.tile([128, 2 * 256], F16)
            nc.sync.dma_start(masks_s[:], masks[:])
            idd = cpool.tile([128, 2 * 128], F16)   # (I | -I)
            id32 = cpool.tile([128, 128], F32)
            nc.sync.dma_start(idd[:], ident16[:])
            nc.sync.dma_start(id32[:], ident32[:])
            id16 = idd[:, 0:128]
            nid16 = idd[:, 128:256]

            # chunk-local column index 0..255 repeated per 256-chunk
            iota = cpool.tile([128, 1024], U32)
            nc.gpsimd.iota(iota[:], pattern=[[0, 4], [1, CHUNK]],
                           base=0, channel_multiplier=0)
            # integer constants as per-partition columns (bitvec ALU ops
            # reject float immediates)
            bitc = cpool.tile([128, 4], U32)
            nc.vector.memset(bitc[:, 0:1], 0xFFFFFF00)
            nc.vector.memset(bitc[:, 1:2], 0x38)
            nc.vector.memset(bitc[:, 2:3], 5)
            nc.vector.memset(bitc[:, 3:4], 0xFF)

            uT = upool.tile([128, NQC], F16)

            # ---------- prep projections (feature-major) + DRAM table ----------
            table = dpool.tile([N, ROWF], F16)
            with tc.tile_pool(name="prep", bufs=1) as prep:
                pts_s = prep.tile([128, 2 * N], F16)
                nc.sync.dma_start(pts_s[:], ptsT[:])
                kT = prep.tile([128, N], F16)
                vT = prep.tile([128, N], F16)
                gT = prep.tile([128, N], F16)
                for col in range(4):
                    cs = slice(col * 512, (col + 1) * 512)
                    acc_k = psm.tile([128, 512], F32, tag="mm")
                    nc.tensor.matmul(acc_k[:], wck_s[:, 0:DIM],
                                     pts_s[:, col * 512:(col + 1) * 512],
                                     start=True, stop=False)
                    nc.tensor.matmul(acc_k[:], wck_s[:, DIM:2 * DIM],
                                     pts_s[:, N + col * 512:N + (col + 1) * 512],
                                     start=False, stop=True)
                    nc.scalar.copy(kT[:, cs], acc_k[:])
                    acc_v = psm.tile([128, 512], F32, tag="mm")
                    nc.tensor.matmul(acc_v[:], wv_s[:, 0:DIM],
                                     pts_s[:, col * 512:(col + 1) * 512],
                                     start=True, stop=False)
                    nc.tensor.matmul(acc_v[:], wv_s[:, DIM:2 * DIM],
                                     pts_s[:, N + col * 512:N + (col + 1) * 512],
                                     start=False, stop=True)
                    nc.scalar.add(vT[:, cs], acc_v[:], b_d2)
                    acc_g = psm.tile([128, 512], F32, tag="mm")
                    nc.tensor.matmul(acc_g[:], wd1_s[:], xyzn4_s[:, cs],
                                     start=True, stop=True)
                    nc.scalar.copy(gT[:, cs], acc_g[:])
                    acc_u = psm.tile([128, 512], F32, tag="mm")
                    nc.tensor.matmul(acc_u[:], wd1_s[:], xyzq4_s[:, cs],
                                     start=True, stop=True)
                    nc.scalar.copy(uT[:, cs], acc_u[:])

                for c in range(N // 128):
                    rs = slice(c * 128, (c + 1) * 128)
                    row_sb = work.tile([128, ROWF], F16, tag="rowsb")
                    for j, src in enumerate((kT, vT, gT)):
                        pt = pss.tile([128, 512], F32, tag="small")
                        ptv = pt[:].bitcast(F16)[:, 0:128]
                        nc.tensor.transpose(ptv, src[:, rs], id16)
                        nc.scalar.copy(row_sb[:, j * 128:(j + 1) * 128], ptv)
                    nc.sync.dma_start(table[rs, :], row_sb[:])

            if stage == 'prep':
                osb0 = work.tile([128, 128], F32, tag="osb")
                nc.vector.tensor_copy(osb0[:], uT[:, 0:128])
                for r in range(16):
                    nc.sync.dma_start(out[r * 128:(r + 1) * 128, :], osb0[:])

            # ---------- per-tile topk ----------
            def topk_tile(t):
                qs = slice(t * QTILE, (t + 1) * QTILE)
                spk = spkp.tile([128, N], U32, tag="spk")
                for half in range(2):
                    sps = psd.tile([128, 1024], F32, tag="dist")
                    for j in range(2):
                        cs = slice(half * 1024 + j * 512, half * 1024 + (j + 1) * 512)
                        nc.tensor.matmul(sps[:, j * 512:(j + 1) * 512],
                                         qx4_s[:, qs], xt4_s[:, cs],
                                         start=True, stop=True)
                    nc.vector.scalar_tensor_tensor(
                        spk[:, half * 1024:(half + 1) * 1024],
                        sps[:].bitcast(U32), bitc[:, 0:1],
                        iota[:].bitcast(U32), ALU.bitwise_and, ALU.bitwise_or)
                spkf = spk[:].bitcast(F32)
                cand = work.tile([128, NCHUNK * 8], F32, tag="cand")
                for c in range(NCHUNK):
                    nc.vector.max(cand[:, c * 8:(c + 1) * 8],
                                  spkf[:, c * CHUNK:(c + 1) * CHUNK])
                winners = work.tile([128, 16], F32, tag="win")
                pos = work.tile([128, 16], U32, tag="pos")
                nc.vector.max(winners[:, 0:8], cand[:])
                nc.vector.max_index(pos[:, 0:8], winners[:, 0:8], cand[:])
                nc.vector.match_replace(cand[:], winners[:, 0:8], cand[:], -3e38)
                nc.vector.max(winners[:, 8:16], cand[:])
                nc.vector.max_index(pos[:, 8:16], winners[:, 8:16], cand[:])
                # global idx = (packed & 0xFF) + (pos//8)*256
                base = work.tile([128, 16], U32, tag="base")
                nc.vector.tensor_scalar(base[:], pos[:], bitc[:, 1:2],
                                        bitc[:, 2:3], ALU.bitwise_and,
                                        ALU.logical_shift_left)
                wid = work.tile([128, 16], U32, tag="wid")
                nc.vector.tensor_scalar(wid[:], winners[:].bitcast(U32),
                                        bitc[:, 3:4], None, ALU.bitwise_and)
                nc.vector.tensor_tensor(wid[:], wid[:], base[:], ALU.add)
                wif = work.tile([128, 16], F16, tag=f"wif{t % 4}")
                nc.vector.tensor_copy(wif[:], wid[:])
                return wif

            # ---------- selector + gather for one block ----------
            def sel_gather(gb, wif_list):
                psel = pss.tile([128, 512], F32, tag="small")
                for t2 in range(2):
                    rhs = work.tile([128, 256], F16, tag="rhs")
                    nc.vector.tensor_tensor(
                        rhs[:].rearrange("p (a b) -> p a b", a=16),
                        wif_list[t2][:].unsqueeze(2).broadcast_to((128, 16, 16)),
                        masks_s[:, t2 * 256:(t2 + 1) * 256]
                        .rearrange("p (a b) -> p a b", a=16),
                        ALU.mult)
                    nc.tensor.matmul(psel[:, 0:256], esel_s[:], rhs[:],
                                     start=(t2 == 0), stop=(t2 == 1))
                idxs = work.tile([128, 256], I16, tag="idxs")
                nc.scalar.copy(idxs[:], psel[:, 0:256])
                gath = gp.tile([128, 3, 4096], F16, tag="gath")
                nc.gpsimd.dma_gather(gath[:], table[:], idxs[:],
                                     num_idxs=4096, num_idxs_reg=4096,
                                     elem_size=ROWF, transpose=True,
                                     single_packet=False)
                return gath

            # ---------- MLP for one block; returns tail state ----------
            def mlp_block(gb, gath):
                qs = slice(gb * QBLK, (gb + 1) * QBLK)
                k2f = gath[:, 0, :]
                vf = gath[:, 1, :]
                g3 = gath[:, 2, :].rearrange("p (a b) -> p a b", a=16)
                ub = uT[:, qs].unsqueeze(1).broadcast_to((128, 16, QBLK))

                # h = relu(u - g) on DVE
                hT = mlp.tile([128, 4096], F16, tag="h")
                h3 = hT[:].rearrange("p (a b) -> p a b", a=16)
                nc.vector.tensor_tensor(h3, ub, g3, ALU.subtract)
                nc.vector.tensor_scalar_max(hT[:], hT[:], 0.0)

                vpos = mlp.tile([128, 4096], F16, tag="vp")
                g1 = mlp.tile([128, 4096], F16, tag="g1")
                expt = mlp.tile([128, 4096], F16, tag="ex")
                for col in range(8):
                    cs = slice(col * 512, (col + 1) * 512)
                    pv = psm.tile([128, 512], F32, tag="mm")
                    nc.tensor.matmul(pv[:], wd2_s[:], hT[:, cs], start=True, stop=False)
                    nc.tensor.matmul(pv[:], id16, vf[:, cs], start=False, stop=True)
                    nc.scalar.copy(vpos[:, cs], pv[:])
                    pg = psm.tile([128, 512], F32, tag="mm")
                    nc.tensor.matmul(pg[:], wm_s[:], hT[:, cs], start=True, stop=False)
                    nc.tensor.matmul(pg[:], id16, k2f[:, cs], start=False, stop=True)
                    nc.scalar.activation(g1[:, cs], pg[:], ACTF.Relu, bias=bg1eff)
                    pe_ = psm.tile([128, 512], F32, tag="mm")
                    nc.tensor.matmul(pe_[:], wg2_s[:], g1[:, cs], start=True, stop=True)
                    nc.scalar.activation(expt[:, cs], pe_[:], ACTF.Exp, bias=b_g2)

                # wp = expt * vpos on DVE
                wp = mlp.tile([128, 4096], F16, tag="wpp")
                nc.vector.tensor_tensor(wp[:], expt[:], vpos[:], ALU.mult)
                return expt, wp

            # ---------- block tail: PE trees + normalize + out ----------
            def block_tail(gb, expt, wp):
                e3 = expt[:].rearrange("p (a b) -> p a b", a=16)
                wp3 = wp[:].rearrange("p (a b) -> p a b", a=16)
                pes = pse.tile([128, 512], F32, tag="tree")
                for a in range(16):
                    nc.tensor.matmul(pes[:, 0:256], id16, e3[:, a, :],
                                     start=(a == 0), stop=False)
                nc.tensor.matmul(pes[:, 0:256], id16, ebt_s[:, 0:256],
                                 start=False, stop=True)
                for a in range(16):
                    nc.tensor.matmul(pes[:, 256:512], id16, wp3[:, a, :],
                                     start=(a == 0), stop=False)
                nc.tensor.matmul(pes[:, 256:512], id16, ebt_s[:, 256:512],
                                 start=False, stop=True)
                rec = mlp.tile([128, QBLK], F32, tag="rec")
                nc.vector.reciprocal(rec[:], pes[:, 0:256])
                res = mlp.tile([128, QBLK], F32, tag="res")
                nc.vector.tensor_tensor(res[:], pes[:, 256:512], rec[:], ALU.mult)
                for t2 in range(2):
                    po = pss.tile([128, 512], F32, tag="small")
                    nc.tensor.transpose(po[:, 0:128],
                                        res[:, t2 * 128:(t2 + 1) * 128], id32[:])
                    osb = work.tile([128, 128], F32, tag="osb")
                    nc.scalar.copy(osb[:], po[:, 0:128])
                    nc.sync.dma_start(
                        out[gb * QBLK + t2 * 128: gb * QBLK + (t2 + 1) * 128, :],
                        osb[:])

            if stage == 'prep':
                nc.compile()
                return nc

            # ---------- software-pipelined main loop ----------
            LOOKAHEAD = 2
            gaths = {}
            for pre in range(LOOKAHEAD):
                w0 = topk_tile(2 * pre)
                w1 = topk_tile(2 * pre + 1)
                gaths[pre] = sel_gather(pre, [w0, w1])

            if stage == 'topk':
                osb0 = work.tile([128, 128], F32, tag="osb")
                nc.vector.tensor_copy(osb0[:, 0:128],
                                      gaths[0][:, 0, 0:128])
                for r in range(16):
                    nc.sync.dma_start(out[r * 128:(r + 1) * 128, :], osb0[:])
                nc.compile()
                return nc

            tail = None
            for gb in range(NBLK):
                if tail is not None:
                    block_tail(*tail)
                nb = gb + LOOKAHEAD
                if nb < NBLK:
                    w0 = topk_tile(2 * nb)
                    w1 = topk_tile(2 * nb + 1)
                    gaths[nb] = sel_gather(nb, [w0, w1])
                expt, wp = mlp_block(gb, gaths.pop(gb))
                tail = (gb, expt, wp)
            block_tail(*tail)

    nc.compile()
    return nc


def _host_prep(inputs):
    """Build the 8 per-core input maps from full inputs (layout prep only)."""
    xyz_q = np.asarray(inputs["xyz_q"], np.float32)
    lat_rep = np.asarray(inputs["lat_rep"], np.float32)
    xyz = np.asarray(inputs["xyz"], np.float32)
    points = np.asarray(inputs["points"], np.float32)
    W_d1 = np.asarray(inputs["W_d1"], np.float32); b_d1 = np.asarray(inputs["b_d1"], np.float32)
    W_d2 = np.asarray(inputs["W_d2"], np.float32); b_d2 = np.asarray(inputs["b_d2"], np.float32)
    W_g1 = np.asarray(inputs["W_g1"], np.float32); b_g1 = np.asarray(inputs["b_g1"], np.float32)
    W_g2 = np.asarray(inputs["W_g2"], np.float32); b_g2 = np.asarray(inputs["b_g2"], np.float32)
    W_kg = np.asarray(inputs["W_kg"], np.float32)
    W_vg = np.asarray(inputs["W_vg"], np.float32)
    W_q = np.asarray(inputs["W_q"], np.float32)
    W_k = np.asarray(inputs["W_k"], np.float32)
    W_v = np.asarray(inputs["W_v"], np.float32)

    # per-batch global-slot constants
    q_attn = lat_rep @ W_q.T                      # [B, DIM]
    k_g = lat_rep @ W_kg.T
    v_g = lat_rep @ W_vg.T
    tg = q_attn - k_g
    g1g = np.maximum(tg @ W_g1.T + b_g1, 0.0)
    logit_g = g1g @ W_g2.T + b_g2
    exp_g = np.exp(logit_g)                       # [B, DIM]
    egv = exp_g * v_g

    # fused weights
    Wck = W_g1 @ W_k                              # [DIM, DI]
    M = W_g1 @ W_d2                               # [DIM, DIM]
    bg1eff = b_g1 + (b_d2 + q_attn) @ W_g1.T      # [B, DIM]

    # constants
    qp = np.arange(128)
    esel = (qp[:, None] % 16 == qp[None, :] % 16).astype(np.float16)  # [q',p]
    masks = np.zeros((2, 128, 256), np.float16)
    g_of = qp // 16                               # q' // 16 in 0..7
    for t in range(2):
        for nb_ in range(16):
            for g in range(16):
                masks[t, :, nb_ * 16 + g] = (g_of == (g - t * 8)).astype(np.float16)
    ident16 = np.concatenate([np.eye(128, dtype=np.float16),
                              -np.eye(128, dtype=np.float16)], axis=1)
    ident32 = np.eye(128, dtype=np.float32)

    wd1_l = np.concatenate([W_d1.T, b_d1[None, :]], axis=0).astype(np.float16)  # [4,128]

    maps = []
    for core in range(8):
        b, h = core // 2, core % 2
        qsl = slice(h * NQC, (h + 1) * NQC)
        xq = xyz_q[b, qsl]                        # [2048, 3]
        xn = xyz[b]                               # [2048, 3]
        qx4 = np.concatenate([2.0 * xq.T, np.ones((1, NQC), np.float32)], axis=0)
        xt4 = np.concatenate([xn.T, -np.sum(xn * xn, axis=1)[None, :]], axis=0)
        xyzq4 = np.concatenate([xq.T, np.ones((1, NQC), np.float32)], axis=0).astype(np.float16)
        xyzn4 = np.concatenate([xn.T, np.zeros((1, N), np.float32)], axis=0).astype(np.float16)
        pT = points[b].T.astype(np.float16)          # [256, N]
        ptsT = np.concatenate([pT[0:128], pT[128:256]], axis=1)  # [128, 2N]
        colv = np.stack([b_d2, bg1eff[b], b_g2], axis=1).astype(np.float32)
        ebt_t = np.concatenate([
            np.broadcast_to(exp_g[b][:, None], (128, QBLK)),
            np.broadcast_to(egv[b][:, None], (128, QBLK))], axis=1).astype(np.float16)
        maps.append({
            "qx4": np.ascontiguousarray(qx4, np.float32),
            "xt4": np.ascontiguousarray(xt4, np.float32),
            "ptsT": np.ascontiguousarray(ptsT),
            "xyzq4": np.ascontiguousarray(xyzq4),
            "xyzn4": np.ascontiguousarray(xyzn4),
            "wck_l": np.ascontiguousarray(np.concatenate(
                [-Wck.T[0:128], -Wck.T[128:256]], axis=1).astype(np.float16)),
            "wv_l": np.ascontiguousarray(np.concatenate(
                [W_v.T[0:128], W_v.T[128:256]], axis=1).astype(np.float16)),
            "wd1_l": np.ascontiguousarray(wd1_l),
            "wd2_l": np.ascontiguousarray(W_d2.T.astype(np.float16)),
            "wm_l": np.ascontiguousarray(M.T.astype(np.float16)),
            "wg2_l": np.ascontiguousarray(W_g2.T.astype(np.float16)),
            "colv": np.ascontiguousarray(colv),
            "ebt": np.ascontiguousarray(ebt_t),
            "esel": np.ascontiguousarray(esel),
            "masks": np.ascontiguousarray(
                np.concatenate([masks[0], masks[1]], axis=1)),
            "ident16": np.ascontiguousarray(ident16),
            "ident32": ident32,
        })
    return maps


def kernel(**inputs):
    if "nc" not in _CACHE:
        _CACHE["nc"] = _build()
    nc = _CACHE["nc"]
    maps = _host_prep(inputs)
    res = run_bass_kernel_spmd(nc, maps, core_ids=list(range(8)))
    _CACHE["res"] = res
    out = np.empty((B, NQ, DIM), np.float32)
    for core in range(8):
        b, h = core // 2, core % 2
        out[b, h * NQC:(h + 1) * NQC, :] = res.results[core]["out"]
    return out
